# revision 17
# baseline (speedup 1.0000x reference)
"""Trainium2 Bass kernel for masked dual-softmax attention.

Reference computation (per batch b, head h, dh=16, H=8, N=1024, D=128):
  q = query @ Wq + bq ; k = key @ Wk + bk ; v = value @ Wv + bv
  S = q_h k_h^T / sqrt(dh)
  attn = 0.5*(softmax(S) + softmax(S masked by adj))
  out = concat_h(attn @ v_h) @ Wo + bo

Sharding: data-parallel over batch, one batch element per NeuronCore (8 cores).

Per-core device algorithm (transposed-score layout; m = key index on
partitions, n = query index on free dim):
  - load host-pretransposed xT = [D, N] inputs; project with head-permuted
    weight tiles so head j of each group lands on SBUF partition quadrant 32j
  - S^T chunks [m=128, n=512] for 2 heads -> one PSUM [128,1024] region
  - one ACT exp per region (PSUM->SBUF bf16); DVE multiplies by the
    transposed adjacency mask (free-dim broadcast over the 2 heads)
  - attn@V via ones-augmented V: lhsT = [1|v_h] gives softmax denominators in
    the same PSUM accumulator row (quadrant-aligned for partition_broadcast)
  - normalize: reciprocal_approx_fast on the sums rows, GPSIMD
    partition_broadcast across the quadrant, one DVE multiply
  - output projection with a packed Wo (0.5 factor folded; zero rows kill
    the sums rows), bias via K=1 ones-row matmul
"""

import sys

if "/opt/trn_rl_repo" not in sys.path:
    sys.path.insert(0, "/opt/trn_rl_repo")

import numpy as np
import ml_dtypes
from contextlib import ExitStack

B, N, D, H, DH = 8, 1024, 128, 8, 16
NCORES = 8
P = 128
NH = 2          # n halves of 512
NHF = N // NH   # 512
MC = 8          # m chunks of 128
G2 = 4          # head groups of 2

_BF16 = ml_dtypes.bfloat16
_CACHE = {}


def _build_nc(debug=False):
    import concourse.bass as bass
    import concourse.tile as tile
    import concourse.mybir as mybir
    from concourse import bacc

    bf16 = mybir.dt.bfloat16
    f32 = mybir.dt.float32
    Exp = mybir.ActivationFunctionType.Exp

    nc = bacc.Bacc("TRN2", target_bir_lowering=False, debug=False,
                   num_devices=NCORES)

    # ---- DRAM I/O -------------------------------------------------------
    xq_d = nc.dram_tensor("xqT", [P, N], mybir.dt.float32r, kind="ExternalInput")
    xk_d = nc.dram_tensor("xkT", [P, N], mybir.dt.float32r, kind="ExternalInput")
    xv_d = nc.dram_tensor("xvT", [P, N], bf16, kind="ExternalInput")
    mask_d = nc.dram_tensor("maskL", [P, MC * NH * NHF], bf16, kind="ExternalInput")
    wqa_d = nc.dram_tensor("wqa", [P, P], mybir.dt.float32r, kind="ExternalInput")
    wqb_d = nc.dram_tensor("wqb", [P, P], mybir.dt.float32r, kind="ExternalInput")
    wka_d = nc.dram_tensor("wka", [P, P], mybir.dt.float32r, kind="ExternalInput")
    wkb_d = nc.dram_tensor("wkb", [P, P], mybir.dt.float32r, kind="ExternalInput")
    wv_d = nc.dram_tensor("wv", [P, P], bf16, kind="ExternalInput")
    wo_d = nc.dram_tensor("wo4", [P, 4 * P], bf16, kind="ExternalInput")  # packed per g2
    bqa_d = nc.dram_tensor("bqa", [P, 1], f32, kind="ExternalInput")  # (bq/4) columns
    bqb_d = nc.dram_tensor("bqb", [P, 1], f32, kind="ExternalInput")
    bka_d = nc.dram_tensor("bka", [P, 1], f32, kind="ExternalInput")
    bkb_d = nc.dram_tensor("bkb", [P, 1], f32, kind="ExternalInput")
    bv_d = nc.dram_tensor("bvr", [1, P], bf16, kind="ExternalInput")  # row
    bo_d = nc.dram_tensor("bor", [1, 4 * P], bf16, kind="ExternalInput")  # bo tiled 4x
    bsel_d = nc.dram_tensor("bsel", [P, P], mybir.dt.float32r, kind="ExternalInput")  # quadrant-sum selector
    out_d = nc.dram_tensor("out", [N, D], f32, kind="ExternalOutput")
    dbg = {}
    if debug:
        for nm, shp in [("d_qa", [P, N]), ("d_ka", [P, N]), ("d_vaug", [P, MC * H * 32]),
                        ("d_eg", [P, N]), ("d_em", [P, N]), ("d_ogsb", [P, NHF]),
                        ("d_rbc", [P, NHF]), ("d_rrec", [P, NHF]), ("d_attnT", [P, NHF])]:
            dbg[nm] = nc.dram_tensor(nm, shp, f32, kind="ExternalOutput")

    with tile.TileContext(nc) as tc, ExitStack() as ctx:
        const = ctx.enter_context(tc.tile_pool(name="const", bufs=1))
        xpool = ctx.enter_context(tc.tile_pool(name="x", bufs=1))
        qkpool = ctx.enter_context(tc.tile_pool(name="qk", bufs=1))
        egp = ctx.enter_context(tc.tile_pool(name="eg", bufs=4))
        emp = ctx.enter_context(tc.tile_pool(name="em", bufs=4))
        atp = ctx.enter_context(tc.tile_pool(name="attnT", bufs=3))
        rscp = ctx.enter_context(tc.tile_pool(name="rsc", bufs=2))
        rbcp = ctx.enter_context(tc.tile_pool(name="rbc", bufs=2))
        osb = ctx.enter_context(tc.tile_pool(name="osb", bufs=2))
        # PSUM: s4 2 banks x2, ogom 1 bank x2, outp 1 bank x2 = 8 banks
        s4p = ctx.enter_context(tc.tile_pool(name="s4", bufs=2, space="PSUM"))
        ogp = ctx.enter_context(tc.tile_pool(name="ogom", bufs=2, space="PSUM"))
        outp = ctx.enter_context(tc.tile_pool(name="outp", bufs=1, space="PSUM"))
        rpsp = ctx.enter_context(tc.tile_pool(name="rps", bufs=1, space="PSUM"))

        # ---- constants / inputs -----------------------------------------
        wqa = const.tile([P, P], mybir.dt.float32r, tag="wqa")
        wqb = const.tile([P, P], mybir.dt.float32r, tag="wqb")
        wka = const.tile([P, P], mybir.dt.float32r, tag="wka")
        wkb = const.tile([P, P], mybir.dt.float32r, tag="wkb")
        wv = const.tile([P, P], bf16, tag="wv")
        wo4 = const.tile([P, 4 * P], bf16, tag="wo4")
        bqa = const.tile([P, 1], f32, tag="bqa")
        bqb = const.tile([P, 1], f32, tag="bqb")
        bka = const.tile([P, 1], f32, tag="bka")
        bkb = const.tile([P, 1], f32, tag="bkb")
        bvr = const.tile([1, P], bf16, tag="bvr")
        bor = const.tile([1, 4 * P], bf16, tag="bor")
        ones1 = const.tile([1, P], bf16, tag="ones1")
        bsel = const.tile([P, P], mybir.dt.float32r, tag="bsel")
        mask_sb = const.tile([P, MC, NH, NHF], bf16, tag="mask")
        for t, d in [(wqa, wqa_d), (wqb, wqb_d), (wka, wka_d), (wkb, wkb_d),
                     (wv, wv_d), (wo4, wo_d), (bqa, bqa_d), (bqb, bqb_d),
                     (bka, bka_d), (bkb, bkb_d), (bvr, bv_d), (bor, bo_d),
                     (bsel, bsel_d)]:
            nc.sync.dma_start(t[:], d.ap())
        mask_dr = mask_d.ap().rearrange("p (a b f) -> p a b f", a=MC, b=NH)
        for _mc in range(MC):
            nc.sync.dma_start(mask_sb[:, _mc, :, :], mask_dr[:, _mc, :, :])
        nc.vector.memset(ones1[:], 1.0)

        xq = xpool.tile([P, N], mybir.dt.float32r, tag="xq")
        xk = xpool.tile([P, N], mybir.dt.float32r, tag="xk")
        xv = xpool.tile([P, N], bf16, tag="xv")
        nc.sync.dma_start(xq[:], xq_d.ap())
        nc.sync.dma_start(xk[:], xk_d.ap())
        nc.sync.dma_start(xv[:], xv_d.ap())

        # ---- projections -------------------------------------------------
        # qT/kT packed tiles: quadrant 32j+d holds head (4t+j) row d
        qk_tiles = {}
        for name, w, x, bias in [("qa", wqa, xq, bqa), ("qb", wqb, xq, bqb),
                                 ("ka", wka, xk, bka), ("kb", wkb, xk, bkb)]:
            ps = s4p.tile([P, N], f32, tag="s4")
            for s in range(NH):
                nc.tensor.matmul(ps[:, s * NHF:(s + 1) * NHF], w[:],
                                 x[:, s * NHF:(s + 1) * NHF],
                                 start=True, stop=True)
            sb_t = qkpool.tile([P, N], mybir.dt.float32r, tag=name)
            nc.vector.tensor_scalar_add(sb_t[:], ps[:], bias[:])
            qk_tiles[name] = sb_t
            if debug and name in ("qa", "ka"):
                dt = qkpool.tile([P, N], f32, tag="dbg" + name)
                nc.vector.tensor_copy(dt[:], sb_t[:])
                nc.sync.dma_start(dbg["d_" + name].ap(), dt[:])

        # v augmented: [P(m), mc, h, 32]; col 0 = ones, 1..16 = v_h, rest 0
        vaug = qkpool.tile([P, MC, H, 32], bf16, tag="vaug")
        nc.vector.memset(vaug[:], 0.0)
        nc.vector.memset(vaug[:, :, :, 0], 1.0)
        for mc in range(MC):
            ps = s4p.tile([P, N], f32, tag="s4")
            nc.tensor.matmul(ps[:, 0:P], ones1[:], bvr[:], start=True, stop=False)
            nc.tensor.matmul(ps[:, 0:P], xv[:, mc * P:(mc + 1) * P], wv[:],
                             start=False, stop=True)
            nc.vector.tensor_copy(
                vaug[:, mc, :, 1:17],
                ps[:, 0:P].rearrange("p (h d) -> p h d", h=H))  # cols 17..31 stay 0

        # ---- main loop ---------------------------------------------------
        for nh in range(NH):
            out_ps = outp.tile([P, NHF], f32, tag="outp")
            nc.tensor.matmul(out_ps[:], ones1[:], bor[:], start=True, stop=False)
            for g2 in range(G2):
                h0 = 2 * g2
                t = "a" if h0 < 4 else "b"
                qT = qk_tiles["q" + t]
                kT = qk_tiles["k" + t]
                q0 = (2 * g2) % 4       # quadrant of h0 within tile t
                ogom = ogp.tile([P, NHF], f32, tag="ogom")
                for mc in range(MC):
                    s4 = s4p.tile([P, N], f32, tag="s4")
                    for i in range(2):  # heads h0, h0+1
                        qq = 32 * (q0 + i)
                        nc.tensor.matmul(
                            s4[:, i * NHF:(i + 1) * NHF],
                            kT[qq:qq + 16, mc * P:(mc + 1) * P],
                            qT[qq:qq + 16, nh * NHF:(nh + 1) * NHF],
                            start=True, stop=True, tile_position=(qq, 0))
                    eg = egp.tile([P, N], bf16, tag="eg")
                    nc.scalar.activation(eg[:], s4[:], Exp)
                    if debug and nh == 0 and g2 == 0 and mc == 0:
                        det = egp.tile([P, N], f32, tag="dbge")
                        nc.vector.tensor_copy(det[:], eg[:])
                        nc.sync.dma_start(dbg["d_eg"].ap(), det[:])
                    em = emp.tile([P, N], bf16, tag="em")
                    msk = mask_sb[:, mc, nh, :]
                    nc.vector.tensor_mul(
                        em[:].rearrange("p (i f) -> p i f", i=2),
                        eg[:].rearrange("p (i f) -> p i f", i=2),
                        msk[:, None, :].broadcast_to([P, 2, NHF]))
                    if debug and nh == 0 and g2 == 0 and mc == 0:
                        dem = emp.tile([P, N], f32, tag="dbgm")
                        nc.vector.tensor_copy(dem[:], em[:])
                        nc.sync.dma_start(dbg["d_em"].ap(), dem[:])
                    # attn @ V with fused denominators
                    for i in range(2):
                        lhs = vaug[:, mc, h0 + i, :]
                        nc.tensor.matmul(
                            ogom[32 * i:32 * i + 32, :], lhs,
                            eg[:, i * NHF:(i + 1) * NHF],
                            start=(mc == 0), stop=(mc == MC - 1),
                            tile_position=(0, 32 * i), skip_group_check=True)
                        nc.tensor.matmul(
                            ogom[64 + 32 * i:64 + 32 * i + 32, :], lhs,
                            em[:, i * NHF:(i + 1) * NHF],
                            start=(mc == 0), stop=(mc == MC - 1),
                            tile_position=(0, 64 + 32 * i), skip_group_check=True)
                # normalize: rows 32q are denominators, rows 32q+1+d data
                ogsb = rscp.tile([P, NHF], f32, tag="rsc")
                nc.vector.tensor_copy(ogsb[:].bitcast(mybir.dt.float32r), ogom[:])
                rps = rpsp.tile([P, NHF], f32, tag="rps", name=f"rps{nh}_{g2}")
                nc.tensor.matmul(rps[:, 0:NHF], bsel[:],
                                 ogsb[:].bitcast(mybir.dt.float32r),
                                 start=True, stop=True)
                rrec = rbcp.tile([P, NHF], f32, tag="rrec")
                nc.vector.reciprocal_approx_fast(rrec[:], rps[:, 0:NHF])
                attnT = atp.tile([P, NHF], bf16, tag="attnT")
                nc.vector.tensor_mul(attnT[:], ogsb[:], rrec[:])
                if debug and nh == 0 and g2 == 0:
                    nc.sync.dma_start(dbg["d_ogsb"].ap(), ogsb[:])
                    nc.sync.dma_start(dbg["d_rbc"].ap(), rrec[:])
                    nc.sync.dma_start(dbg["d_rrec"].ap(), rrec[:])
                    dat = atp.tile([P, NHF], f32, tag="dbga")
                    nc.vector.tensor_copy(dat[:], attnT[:])
                    nc.sync.dma_start(dbg["d_attnT"].ap(), dat[:])
                # output projection accumulation
                for nt in range(4):
                    nc.tensor.matmul(out_ps[:, nt * P:(nt + 1) * P],
                                     attnT[:, nt * P:(nt + 1) * P],
                                     wo4[:, g2 * P:(g2 + 1) * P],
                                     start=False,
                                     stop=(g2 == G2 - 1 and nt == 3))
            # store
            ob = osb.tile([P, NHF], f32, tag="osb")
            nc.vector.tensor_copy(ob[:], out_ps[:])
            dst = out_d.ap().rearrange("(x t p) d -> x p t d", x=NH, t=4, p=P)[nh]
            nc.sync.dma_start(dst, ob[:].rearrange("p (t d) -> p t d", t=4))

    nc.compile()
    return nc


def _host_prep(query, key, value, adj_mask, Wq, bq, Wk, bk, Wv, bv, Wo, bo):
    """Build the per-core input maps (host-side layout transforms only)."""
    f32 = np.float32
    query = np.asarray(query, f32)
    key = np.asarray(key, f32)
    value = np.asarray(value, f32)
    Wq = np.asarray(Wq, f32); Wk = np.asarray(Wk, f32)
    Wv = np.asarray(Wv, f32); Wo = np.asarray(Wo, f32)
    bq = np.asarray(bq, f32); bk = np.asarray(bk, f32)
    bv = np.asarray(bv, f32); bo = np.asarray(bo, f32)
    adj = np.asarray(adj_mask)

    scale = 1.0 / np.sqrt(np.float32(DH))

    def pack_w(Wm):
        # head-permuted weight columns: tile t, quadrant j <- head 4t+j
        out = []
        for t in range(2):
            wt = np.zeros((P, P), f32)
            for j in range(4):
                h = 4 * t + j
                wt[:, 32 * j:32 * j + 16] = Wm[:, DH * h:DH * (h + 1)]
            out.append(wt)
        return out

    wqa, wqb = [w * scale for w in pack_w(Wq)]
    wka, wkb = pack_w(Wk)

    # packed bias columns (quadrant layout), one per projection tile
    def pack_b2(bvec, s):
        cols = []
        for t in range(2):
            col = np.zeros((P, 1), f32)
            for j in range(4):
                h = 4 * t + j
                col[32 * j:32 * j + 16, 0] = bvec[DH * h:DH * (h + 1)] * s
            cols.append(col)
        return cols

    bqa, bqb = pack_b2(bq, scale)
    bka, bkb = pack_b2(bk, 1.0)

    # packed Wo per head-group g2: rows 32q+1+d ; 0.5 factor folded
    wo4 = np.zeros((P, 4 * P), f32)
    for g2 in range(G2):
        for i in range(2):      # g quadrants 0,1
            h = 2 * g2 + i
            wo4[32 * i + 1:32 * i + 17, g2 * P:(g2 + 1) * P] = 0.5 * Wo[DH * h:DH * (h + 1), :]
        for i in range(2):      # m quadrants 2,3
            h = 2 * g2 + i
            wo4[64 + 32 * i + 1:64 + 32 * i + 17, g2 * P:(g2 + 1) * P] = 0.5 * Wo[DH * h:DH * (h + 1), :]

    # transposed mask, device layout [p, mc, nh, nhf]
    maskT = adj.T.astype(f32)  # [m, n]
    maskL = maskT.reshape(MC, P, NH, NHF).transpose(1, 0, 2, 3).reshape(P, -1)

    bsel = np.zeros((P, P), f32)
    for r in range(P):
        bsel[32 * (r // 32), r] = 1.0

    shared = {
        "bsel": bsel,
        "maskL": maskL.astype(_BF16),
        "wqa": wqa, "wqb": wqb,
        "wka": wka, "wkb": wkb,
        "wv": Wv.astype(_BF16), "wo4": wo4.astype(_BF16),
        "bvr": bv.reshape(1, P).astype(_BF16),
        "bor": np.tile(bo.reshape(1, P), (1, 4)).astype(_BF16),
    }
    in_maps = []
    for b in range(B):
        m = dict(shared)
        m["xqT"] = np.ascontiguousarray(query[b].T)
        m["xkT"] = np.ascontiguousarray(key[b].T)
        m["xvT"] = np.ascontiguousarray(value[b].T).astype(_BF16)
        # biases: heads 0-3 use tile a, 4-7 tile b -> but both tiles share
        # one bias input each; pass per-tile columns
        m["bqa"] = bqa
        m["bqb"] = bqb
        m["bka"] = bka
        m["bkb"] = bkb
        in_maps.append(m)
    return in_maps


def kernel(**inputs):
    if "nc" not in _CACHE:
        _CACHE["nc"] = _build_nc()
    nc = _CACHE["nc"]

    from concourse.bass_utils import run_bass_kernel_spmd

    in_maps = _host_prep(**inputs)
    res = run_bass_kernel_spmd(nc, in_maps, core_ids=list(range(NCORES)))
    out = np.stack([res.results[c]["out"] for c in range(NCORES)], axis=0)
    return out.astype(np.float32)
